# revision 34
# baseline (speedup 1.0000x reference)
"""AttentionAggregator Trainium2 kernel (8-core SPMD, data-parallel over nodes).

Math (per node b with neighbors n):
  x_att   = lrelu_.01(x @ W_att);  neib_att = lrelu_.01(neibs @ W_att)
  e[b,n]  = lrelu_.2(x_att[b]@a_x + neib_att[b,n]@a_n)
  att     = softmax_n(e)
  agg[b]  = sum_n att[b,n] * neibs[b,n]
  out     = relu([x@W_fcx, agg@W_fcn])

v3 design (vs the PE-transpose baseline):
  - Host pre-builds TWO fp16 layouts of neibs per core: neT (transposed
    [D, rows] — feeds score matmuls directly, no PE transposes) and neN
    (rows-grouped-by-128 natural [p, (tile, d)] — feeds the attention-apply
    matmuls).  Both DMA at 8KB/partition-line per block.
  - x is pre-transposed on host (xT [D, B]) and persists in SBUF: serves as
    lhsT for both the x-score matmul and the x@W_fcx output matmul.
  - Scores: 258-col relu-pair decomposition of a.lrelu(W z) (exact), drained
    by DVE scalar_tensor_tensor(max0, *cful, accum) / ACT-relu+GPSIMD-reduce.
  - Softmax normalization without cross-partition shuffles: per-node sums,
    reciprocal-broadcast and x-score broadcast are done with tiny constant
    selector matmuls on the PE (psel4/bsel4/bselx).
"""
import warnings
warnings.filterwarnings("ignore")
import numpy as np
from contextlib import ExitStack

import concourse.bass as bass
import concourse.tile as tile
from concourse import bacc, mybir
from concourse.bass_utils import run_bass_kernel_spmd

F32 = mybir.dt.float32
F16 = mybir.dt.float16
AF = mybir.ActivationFunctionType
ALU = mybir.AluOpType
AX = mybir.AxisListType

N_CORES = 8
B_FULL, NB, D, H, O = 20000, 32, 128, 256, 128
HW6 = 2 * H // 2 + 2  # 258 score columns

# Test-harness knobs (ignored by the grading harness, which calls kernel()
# directly): set TRACE_OPTS["trace"]=True to capture an NTFF profile; the
# BassKernelResults of the last run lands in LAST_RESULT[0].
TRACE_OPTS = {}
LAST_RESULT = [None]

# Drain scheduling: score PSUM tiles are consumed via an ACT relu pass over a
# 4-tile PSUM quad into SBUF fp16, then per-quad GROUPED tensor_reduce ops on
# DVE ([128,4,seg]->[128,4], one instruction per sign-segment, no accumulator
# read).  Odd tail tiles fall back to a direct DVE scalar_tensor_tensor drain.
N_QUADS = 8  # ACT-relu quads per 32-tile block (4 tiles each); rest DVE-direct


def _score_weights(W_att: np.ndarray, a_half: np.ndarray):
    """Build the 258-column relu-pair score weight matrix. Returns (W6, split)."""
    pos = np.where(a_half >= 0)[0]
    neg = np.where(a_half < 0)[0]
    Wabs = W_att * np.abs(a_half)[None, :]
    w_d = (W_att @ a_half).astype(np.float64)
    seg1 = np.concatenate([0.99 * Wabs[:, pos], 0.01 * w_d[:, None]], axis=1)
    seg2 = np.concatenate([0.99 * Wabs[:, neg], -0.01 * w_d[:, None]], axis=1)
    W6 = np.concatenate([seg1, seg2], axis=1).astype(np.float32)
    return W6, seg1.shape[1]


def _blocks(bc):
    out = []
    o = 0
    while o < bc:
        f = min(128, bc - o)
        assert f * NB % 128 == 0
        out.append((o, f))
        o += f
    return out


_PROG_CACHE = {}


def _build_program(bc, split_n, split_x, n_cores=N_CORES):
    key = (bc, split_n, split_x, n_cores, N_QUADS)
    if key in _PROG_CACHE:
        return _PROG_CACHE[key]

    nc = bacc.Bacc("TRN2", target_bir_lowering=False, debug=False,
                   num_devices=n_cores)

    R = bc * NB  # neighbor rows per core
    neT_d = nc.dram_tensor("neT", [D, R], F16, kind="ExternalInput").ap()
    neN_d = nc.dram_tensor("neN", [128, R], F16, kind="ExternalInput").ap()
    xT_d = nc.dram_tensor("xT", [D, bc], F16, kind="ExternalInput").ap()
    w6n_d = nc.dram_tensor("w6n", [D, HW6], F16, kind="ExternalInput").ap()
    w6x_d = nc.dram_tensor("w6x", [D, HW6], F16, kind="ExternalInput").ap()
    cfx_d = nc.dram_tensor("cfx", [128, HW6], F16, kind="ExternalInput").ap()
    wfcx_d = nc.dram_tensor("wfcx", [D, O], F16, kind="ExternalInput").ap()
    wfcn_d = nc.dram_tensor("wfcn", [D, O], F16, kind="ExternalInput").ap()
    psel4_d = nc.dram_tensor("psel4", [128, 4], F32, kind="ExternalInput").ap()
    bsel4_d = nc.dram_tensor("bsel4", [4, 128], F32, kind="ExternalInput").ap()
    bselx_d = nc.dram_tensor("bselx", [128, 128], F32, kind="ExternalInput").ap()
    selm_d = nc.dram_tensor("selm", [128, NB], F32, kind="ExternalInput").ap()
    mask32_d = nc.dram_tensor("mask32", [128, 256], F16, kind="ExternalInput").ap()
    out_d = nc.dram_tensor("out", [bc, 2 * O], F32, kind="ExternalOutput").ap()

    with tile.TileContext(nc) as tc, ExitStack() as ctx:
        consts = ctx.enter_context(tc.tile_pool(name="consts", bufs=1))
        netp = ctx.enter_context(tc.tile_pool(name="netp", bufs=3))
        nenp = ctx.enter_context(tc.tile_pool(name="nenp", bufs=3))
        scrp = ctx.enter_context(tc.tile_pool(name="scrp", bufs=4))
        smallp = ctx.enter_context(tc.tile_pool(name="smallp", bufs=3))
        awp = ctx.enter_context(tc.tile_pool(name="awp", bufs=3))
        outp = ctx.enter_context(tc.tile_pool(name="outp", bufs=2))
        ps_tri = ctx.enter_context(tc.tile_pool(name="ps_tri", bufs=2, space="PSUM"))
        ps_misc = ctx.enter_context(tc.tile_pool(name="ps_misc", bufs=1, space="PSUM"))
        ps_af = ctx.enter_context(tc.tile_pool(name="ps_af", bufs=1, space="PSUM"))

        w6n = consts.tile([D, HW6], F16)
        w6x = consts.tile([D, HW6], F16)
        cfx = consts.tile([128, HW6], F16)
        wfcx = consts.tile([D, O], F16)
        wfcn = consts.tile([D, O], F16)
        psel4 = consts.tile([128, 4], F32)
        bsel4 = consts.tile([4, 128], F32)
        bselx = consts.tile([128, 128], F32)
        selm = consts.tile([128, NB], F32)
        mask32 = consts.tile([128, 256], F16)
        xt_all = consts.tile([D, bc], F16)
        for t, d in [(w6n, w6n_d), (w6x, w6x_d), (cfx, cfx_d),
                     (wfcx, wfcx_d), (wfcn, wfcn_d), (psel4, psel4_d),
                     (bsel4, bsel4_d), (bselx, bselx_d), (selm, selm_d),
                     (mask32, mask32_d), (xt_all, xT_d)]:
            nc.sync.dma_start(t[:], d)

        def phaseA(boff, F):
            """DMA loads, score matmuls + drains, x-side score."""
            T = F * NB // 128
            rbase = boff * NB

            netb = netp.tile([128, 32 * D], F16, tag="net")
            nc.sync.dma_start(netb[:, :T * D], neT_d[:, rbase:rbase + 128 * T])
            nenb = nenp.tile([128, 32 * D], F16, tag="nen")
            nc.sync.dma_start(nenb[:, :T * D], neN_d[:, rbase:rbase + 128 * T])

            scolP = smallp.tile([128, NB], F32, tag="scolP")
            scolN = smallp.tile([128, NB], F32, tag="scolN")
            nc.gpsimd.memset(scolN[:, :T], 0.0)

            # Score tiles (x-score rides slot 0 of the first triple): PSUM
            # triples relu'd by ACT into a 3-slot fp16 buffer; DVE does one
            # grouped tensor_reduce per sign-segment.
            s1 = split_n
            sxP = smallp.tile([128, 1], F32, tag="sxP")
            sxN = smallp.tile([128, 1], F32, tag="sxN")
            slot = 0  # slot 0 of the logical slot stream is the x-score
            while slot < T + 1:
                c = min(3, T + 1 - slot)
                qp = ps_tri.tile([128, 1536], F32, tag="qp")
                qv = qp[:].rearrange("p (g c) -> p g c", c=512)
                for g in range(c):
                    if slot + g == 0:
                        if F < 128:
                            nc.vector.memset(qv[64:, 0, :HW6], 0.0)
                        nc.tensor.matmul(qv[:F, 0, :HW6],
                                         xt_all[:, boff:boff + F], w6x[:],
                                         start=True, stop=True)
                    else:
                        t = slot + g - 1
                        nc.tensor.matmul(qv[:, g, :HW6],
                                         netb[:, t * D:(t + 1) * D],
                                         w6n[:], start=True, stop=True)
                scr = scrp.tile([128, 3 * HW6], F16, tag="scr")
                scrv = scr[:].rearrange("p (g c) -> p g c", c=HW6)
                nc.scalar.activation(scrv[:, :c, :], qv[:, :c, :HW6], AF.Relu)
                if slot == 0:
                    nc.vector.tensor_reduce(
                        sxP[:, :], scrv[:, 0, :split_x], axis=AX.X, op=ALU.add)
                    nc.vector.tensor_reduce(
                        sxN[:, :], scrv[:, 0, split_x:HW6], axis=AX.X,
                        op=ALU.add)
                    if c > 1:
                        nc.vector.tensor_reduce(
                            scolP[:, 0:c - 1], scrv[:, 1:c, :s1], axis=AX.X,
                            op=ALU.add)
                        nc.vector.tensor_reduce(
                            scolN[:, 0:c - 1], scrv[:, 1:c, s1:HW6],
                            axis=AX.X, op=ALU.add)
                else:
                    t = slot - 1
                    nc.vector.tensor_reduce(
                        scolP[:, t:t + c], scrv[:, :c, :s1], axis=AX.X,
                        op=ALU.add)
                    nc.vector.tensor_reduce(
                        scolN[:, t:t + c], scrv[:, :c, s1:HW6], axis=AX.X,
                        op=ALU.add)
                slot += c
            sx = smallp.tile([128, 1], F32, tag="sx")
            nc.gpsimd.tensor_tensor(sx[:, :], sxP[:, :], sxN[:, :],
                                    op=ALU.subtract)

            return dict(nenb=nenb, T=T, F=F, boff=boff, scolP=scolP,
                        scolN=scolN, sx=sx)

        def phaseB(st):
            """Softmax (unnormalized exp + reciprocal-Z) and attention weights."""
            T, F = st["T"], st["F"]
            scolP, scolN, sx = st["scolP"], st["scolN"], st["sx"]

            # sxs[p,t] = sx[4t + p//32] via selector matmul
            Rm = smallp.tile([128, NB], F32, tag="Rm")
            nc.gpsimd.tensor_tensor(
                Rm[:, :T], sx[:, 0:1].broadcast_to([128, T]), selm[:, :T],
                op=ALU.mult)
            sxs_ps = ps_misc.tile([128, NB], F32, tag="misc")
            nc.tensor.matmul(sxs_ps[:, :T], bselx[:], Rm[:, :T],
                             start=True, stop=True)

            z0 = smallp.tile([128, NB], F32, tag="z0")
            nc.gpsimd.tensor_tensor(z0[:, :T], scolP[:, :T], scolN[:, :T],
                                    op=ALU.subtract)
            z = smallp.tile([128, NB], F32, tag="z")
            nc.vector.tensor_tensor(z[:, :T], z0[:, :T], sxs_ps[:, :T],
                                    op=ALU.add)
            zl = smallp.tile([128, NB], F32, tag="zl")
            nc.vector.scalar_tensor_tensor(zl[:, :T], z[:, :T], 0.2, z[:, :T],
                                           op0=ALU.mult, op1=ALU.max)
            ew = smallp.tile([128, NB], F32, tag="ew")
            nc.scalar.activation(ew[:, :T], zl[:, :T], AF.Exp)

            # Z per node, reciprocal, broadcast back to [p, t]
            zt_ps = ps_misc.tile([128, NB], F32, tag="misc")
            nc.tensor.matmul(zt_ps[:4, :T], psel4[:], ew[:, :T],
                             start=True, stop=True)
            rz4 = smallp.tile([4, NB], F32, tag="rz4")
            nc.vector.reciprocal(rz4[:4, :T], zt_ps[:4, :T])
            rzf_ps = ps_misc.tile([128, NB], F32, tag="misc")
            nc.tensor.matmul(rzf_ps[:, :T], bsel4[:4, :], rz4[:4, :T],
                             start=True, stop=True)
            ewn = smallp.tile([128, NB], F32, tag="ewn")
            nc.vector.tensor_tensor(ewn[:, :T], ew[:, :T], rzf_ps[:, :T],
                                    op=ALU.mult)

            # Attention weights for the swapped agg matmuls: per 8-tile group
            # awG[:, t, m] = ewn[p, t] iff node-within-group m == 4*(t%8)+p//32.
            awG = awp.tile([128, NB * 32], F16, tag="awG")
            awGv = awG[:].rearrange("p (t m) -> p t m", m=32)
            ngrp = (T + 7) // 8
            for g in range(ngrp):
                jc = min(8, T - 8 * g)
                nc.gpsimd.tensor_tensor(
                    awGv[:, 8 * g:8 * g + jc, :],
                    ewn[:, 8 * g:8 * g + jc].unsqueeze(2).broadcast_to(
                        [128, jc, 32]),
                    mask32[:].rearrange("p (j m) -> p j m", m=32)[:, :jc, :],
                    op=ALU.mult)
            st["awG"] = awG

        def phaseC(st):
            """Attention apply (agg, aw-stationary), transpose, output, store."""
            nenb, awG = st["nenb"], st["awG"]
            T, F, boff = st["T"], st["F"], st["boff"]
            nen_v = nenb[:].rearrange("p (t d) -> p t d", d=D)
            awGv = awG[:].rearrange("p (t m) -> p t m", m=32)
            af1 = ps_af.tile([128, 2 * O], F32, tag="af")
            aggn_ps = af1[:, 0:128]
            ngrp = (T + 7) // 8
            for g in range(ngrp):
                m = min(32, F - 32 * g)
                jc = min(8, T - 8 * g)
                for j in range(jc):
                    nc.tensor.matmul(aggn_ps[32 * g:32 * g + m, :],
                                     awGv[:, 8 * g + j, :m],
                                     nen_v[:, 8 * g + j, :],
                                     start=(j == 0), stop=(j == jc - 1),
                                     tile_position=(0, 32 * g))
            Fp = (F + 15) // 16 * 16
            aggn = awp.tile([128, 128], F16, tag="aggn")
            if Fp > F:
                nc.gpsimd.memset(aggn[64:, :], 0.0)
            nc.vector.tensor_copy(aggn[:F, :], aggn_ps[:F, :])
            aggt = awp.tile([D, 128], F16, tag="aggt")
            nc.sync.dma_start(aggt[:, :Fp], aggn[:Fp, :], transpose=True)

            af2 = ps_af.tile([128, 2 * O], F32, tag="af")
            fc_ps = af2
            nc.tensor.matmul(fc_ps[:F, 0:O], xt_all[:, boff:boff + F], wfcx[:],
                             start=True, stop=True)
            nc.tensor.matmul(fc_ps[:F, O:2 * O], aggt[:, :F], wfcn[:],
                             start=True, stop=True)
            out_sb = outp.tile([128, 2 * O], F32, tag="out")
            nc.scalar.activation(out_sb[:F, :], fc_ps[:F, :], AF.Relu)
            nc.sync.dma_start(out_d[boff:boff + F, :], out_sb[:F, :])

        prev = None
        for (boff, F) in _blocks(bc):
            st = phaseA(boff, F)
            if prev is not None:
                phaseC(prev)
            phaseB(st)
            prev = st
        phaseC(prev)

    nc.compile()
    _PROG_CACHE[key] = nc
    return nc


def kernel(x, neibs, W_att, W_fcx, W_fcn, a, n_cores=N_CORES):
    x = np.asarray(x, dtype=np.float32)
    neibs = np.asarray(neibs, dtype=np.float32)
    W_att = np.asarray(W_att, dtype=np.float32)
    W_fcx = np.asarray(W_fcx, dtype=np.float32)
    W_fcn = np.asarray(W_fcn, dtype=np.float32)
    a = np.asarray(a, dtype=np.float32)

    B = x.shape[0]
    bc = B // n_cores
    a_x, a_n = a[:H, 0], a[H:, 0]
    w6x_np, split_x = _score_weights(W_att, a_x)
    w6n_np, split_n = _score_weights(W_att, a_n)

    nc = _build_program(bc, split_n, split_x, n_cores)

    def cful(split, rep=1):
        v = np.concatenate([np.ones(split), -np.ones(HW6 - split)])
        v = np.tile(v, rep)
        return np.repeat(v[None, :].astype(np.float16), 128, axis=0)

    p = np.arange(128)
    psel4_np = np.equal.outer(p // 32, np.arange(4)).astype(np.float32)
    bsel4_np = np.equal.outer(np.arange(4), p // 32).astype(np.float32)
    bselx_np = np.equal.outer(p % 4, p // 32).astype(np.float32)
    selm_np = np.equal.outer(p // 4, np.arange(NB)).astype(np.float32)
    pj = 4 * np.arange(8)[None, :, None] + (p // 32)[:, None, None]
    mask32_np = (pj == np.arange(32)[None, None, :]).astype(np.float16).reshape(128, 256)

    shared = {
        "w6n": w6n_np.astype(np.float16), "w6x": w6x_np.astype(np.float16),
        "cfx": cful(split_x),
        "wfcx": W_fcx.astype(np.float16), "wfcn": W_fcn.astype(np.float16),
        "psel4": psel4_np, "bsel4": bsel4_np, "bselx": bselx_np,
        "selm": selm_np, "mask32": mask32_np,
    }

    rows_c = bc * NB
    tiles_c = rows_c // 128
    in_maps = []
    for c in range(n_cores):
        sl = neibs[c * rows_c:(c + 1) * rows_c]
        neT_np = np.ascontiguousarray(sl.T).astype(np.float16)
        neN_np = np.ascontiguousarray(
            sl.reshape(tiles_c, 128, D).transpose(1, 0, 2).reshape(128, rows_c)
        ).astype(np.float16)
        xT_np = np.ascontiguousarray(x[c * bc:(c + 1) * bc].T).astype(np.float16)
        in_maps.append({
            "neT": neT_np, "neN": neN_np, "xT": xT_np, **shared,
        })
    res = run_bass_kernel_spmd(nc, in_maps, core_ids=list(range(n_cores)),
                               **TRACE_OPTS)
    LAST_RESULT[0] = res
    return np.concatenate([res.results[c]["out"] for c in range(n_cores)], axis=0)


# revision 35
# speedup vs baseline: 1.0800x; 1.0800x over previous
"""AttentionAggregator Trainium2 kernel (8-core SPMD, data-parallel over nodes).

Math (per node b with neighbors n):
  x_att   = lrelu_.01(x @ W_att);  neib_att = lrelu_.01(neibs @ W_att)
  e[b,n]  = lrelu_.2(x_att[b]@a_x + neib_att[b,n]@a_n)
  att     = softmax_n(e)
  agg[b]  = sum_n att[b,n] * neibs[b,n]
  out     = relu([x@W_fcx, agg@W_fcn])

v3 design (vs the PE-transpose baseline):
  - Host pre-builds TWO fp16 layouts of neibs per core: neT (transposed
    [D, rows] — feeds score matmuls directly, no PE transposes) and neN
    (rows-grouped-by-128 natural [p, (tile, d)] — feeds the attention-apply
    matmuls).  Both DMA at 8KB/partition-line per block.
  - x is pre-transposed on host (xT [D, B]) and persists in SBUF: serves as
    lhsT for both the x-score matmul and the x@W_fcx output matmul.
  - Scores: 258-col relu-pair decomposition of a.lrelu(W z) (exact), drained
    by DVE scalar_tensor_tensor(max0, *cful, accum) / ACT-relu+GPSIMD-reduce.
  - Softmax normalization without cross-partition shuffles: per-node sums,
    reciprocal-broadcast and x-score broadcast are done with tiny constant
    selector matmuls on the PE (psel4/bsel4/bselx).
"""
import warnings
warnings.filterwarnings("ignore")
import numpy as np
from contextlib import ExitStack

import concourse.bass as bass
import concourse.tile as tile
from concourse import bacc, mybir
from concourse.bass_utils import run_bass_kernel_spmd

F32 = mybir.dt.float32
F16 = mybir.dt.float16
AF = mybir.ActivationFunctionType
ALU = mybir.AluOpType
AX = mybir.AxisListType

N_CORES = 8
B_FULL, NB, D, H, O = 20000, 32, 128, 256, 128
HW6 = 2 * H // 2 + 2  # 258 score columns

# Test-harness knobs (ignored by the grading harness, which calls kernel()
# directly): set TRACE_OPTS["trace"]=True to capture an NTFF profile; the
# BassKernelResults of the last run lands in LAST_RESULT[0].
TRACE_OPTS = {}
LAST_RESULT = [None]

# Drain scheduling: score PSUM tiles are consumed via an ACT relu pass over a
# 4-tile PSUM quad into SBUF fp16, then per-quad GROUPED tensor_reduce ops on
# DVE ([128,4,seg]->[128,4], one instruction per sign-segment, no accumulator
# read).  Odd tail tiles fall back to a direct DVE scalar_tensor_tensor drain.
N_QUADS = 8  # ACT-relu quads per 32-tile block (4 tiles each); rest DVE-direct


def _score_weights(W_att: np.ndarray, a_half: np.ndarray):
    """Build the 258-column relu-pair score weight matrix. Returns (W6, split)."""
    pos = np.where(a_half >= 0)[0]
    neg = np.where(a_half < 0)[0]
    Wabs = W_att * np.abs(a_half)[None, :]
    w_d = (W_att @ a_half).astype(np.float64)
    seg1 = np.concatenate([0.99 * Wabs[:, pos], 0.01 * w_d[:, None]], axis=1)
    seg2 = np.concatenate([0.99 * Wabs[:, neg], -0.01 * w_d[:, None]], axis=1)
    W6 = np.concatenate([seg1, seg2], axis=1).astype(np.float32)
    return W6, seg1.shape[1]


def _blocks(bc):
    out = []
    o = 0
    while o < bc:
        f = min(128, bc - o)
        assert f * NB % 128 == 0
        out.append((o, f))
        o += f
    return out


_PROG_CACHE = {}


def _build_program(bc, split_n, split_x, n_cores=N_CORES):
    key = (bc, split_n, split_x, n_cores, N_QUADS)
    if key in _PROG_CACHE:
        return _PROG_CACHE[key]

    nc = bacc.Bacc("TRN2", target_bir_lowering=False, debug=False,
                   num_devices=n_cores)

    R = bc * NB  # neighbor rows per core
    neT_d = nc.dram_tensor("neT", [D, R], F16, kind="ExternalInput").ap()
    neN_d = nc.dram_tensor("neN", [128, R], F16, kind="ExternalInput").ap()
    xT_d = nc.dram_tensor("xT", [D, bc], F16, kind="ExternalInput").ap()
    w6n_d = nc.dram_tensor("w6n", [D, HW6], F16, kind="ExternalInput").ap()
    w6x_d = nc.dram_tensor("w6x", [D, HW6], F16, kind="ExternalInput").ap()
    cfn_d = nc.dram_tensor("cfn", [128, HW6], F16, kind="ExternalInput").ap()
    cfx_d = nc.dram_tensor("cfx", [128, HW6], F16, kind="ExternalInput").ap()
    wfcx_d = nc.dram_tensor("wfcx", [D, O], F16, kind="ExternalInput").ap()
    wfcn_d = nc.dram_tensor("wfcn", [D, O], F16, kind="ExternalInput").ap()
    psel4_d = nc.dram_tensor("psel4", [128, 4], F32, kind="ExternalInput").ap()
    bsel4_d = nc.dram_tensor("bsel4", [4, 128], F32, kind="ExternalInput").ap()
    bselx_d = nc.dram_tensor("bselx", [128, 128], F32, kind="ExternalInput").ap()
    selm_d = nc.dram_tensor("selm", [128, NB], F32, kind="ExternalInput").ap()
    mask4_d = nc.dram_tensor("mask4", [128, 4], F16, kind="ExternalInput").ap()
    out_d = nc.dram_tensor("out", [bc, 2 * O], F32, kind="ExternalOutput").ap()

    with tile.TileContext(nc) as tc, ExitStack() as ctx:
        consts = ctx.enter_context(tc.tile_pool(name="consts", bufs=1))
        netp = ctx.enter_context(tc.tile_pool(name="netp", bufs=3))
        nenp = ctx.enter_context(tc.tile_pool(name="nenp", bufs=3))
        scrp = ctx.enter_context(tc.tile_pool(name="scrp", bufs=4))
        smallp = ctx.enter_context(tc.tile_pool(name="smallp", bufs=3))
        awp = ctx.enter_context(tc.tile_pool(name="awp", bufs=3))
        outp = ctx.enter_context(tc.tile_pool(name="outp", bufs=2))
        ps_sc = ctx.enter_context(tc.tile_pool(name="ps_sc", bufs=1, space="PSUM"))
        ps_pair = ctx.enter_context(tc.tile_pool(name="ps_pair", bufs=2, space="PSUM"))
        ps_misc = ctx.enter_context(tc.tile_pool(name="ps_misc", bufs=1, space="PSUM"))
        ps_agg = ctx.enter_context(tc.tile_pool(name="ps_agg", bufs=1, space="PSUM"))
        ps_fc = ctx.enter_context(tc.tile_pool(name="ps_fc", bufs=1, space="PSUM"))

        w6n = consts.tile([D, HW6], F16)
        w6x = consts.tile([D, HW6], F16)
        cfn = consts.tile([128, HW6], F16)
        cfx = consts.tile([128, HW6], F16)
        wfcx = consts.tile([D, O], F16)
        wfcn = consts.tile([D, O], F16)
        psel4 = consts.tile([128, 4], F32)
        bsel4 = consts.tile([4, 128], F32)
        bselx = consts.tile([128, 128], F32)
        selm = consts.tile([128, NB], F32)
        mask4 = consts.tile([128, 4], F16)
        xt_all = consts.tile([D, bc], F16)
        for t, d in [(w6n, w6n_d), (w6x, w6x_d), (cfn, cfn_d), (cfx, cfx_d),
                     (wfcx, wfcx_d), (wfcn, wfcn_d), (psel4, psel4_d),
                     (bsel4, bsel4_d), (bselx, bselx_d), (selm, selm_d),
                     (mask4, mask4_d), (xt_all, xT_d)]:
            nc.sync.dma_start(t[:], d)

        def phaseA(boff, F):
            """DMA loads, score matmuls + drains, x-side score."""
            T = F * NB // 128
            rbase = boff * NB

            netb = netp.tile([128, 32 * D], F16, tag="net")
            nc.sync.dma_start(netb[:, :T * D], neT_d[:, rbase:rbase + 128 * T])
            nenb = nenp.tile([128, 32 * D], F16, tag="nen")
            nc.sync.dma_start(nenb[:, :T * D], neN_d[:, rbase:rbase + 128 * T])

            scolP = smallp.tile([128, NB], F32, tag="scolP")
            scolN = smallp.tile([128, NB], F32, tag="scolN")
            nc.gpsimd.memset(scolN[:, :T], 0.0)

            # x-side score (drain on DVE)
            xs_ps = ps_sc.tile([128, HW6], F32, tag="sc")
            nc.tensor.matmul(xs_ps[:F, :], xt_all[:, boff:boff + F], w6x[:],
                             start=True, stop=True)
            xscr = scrp.tile([128, HW6], F16, tag="xscr")
            sx = smallp.tile([128, 1], F32, tag="sx")
            nc.vector.scalar_tensor_tensor(
                xscr[:F, :], xs_ps[:F, :], 0.0, cfx[:F, :],
                op0=ALU.max, op1=ALU.mult, accum_out=sx[:F, :])

            # neighbor score tiles: PSUM pairs relu'd by ACT into halves of a
            # 4-tile fp16 buffer; DVE then does one grouped tensor_reduce per
            # sign-segment covering 4 tiles.  Odd tail tile drains direct.
            s1 = split_n
            t = 0
            pend = None

            def reduce_group(scr4, t0, G):
                v = scr4[:].rearrange("p (g c) -> p g c", c=HW6)
                nc.vector.tensor_reduce(
                    scolP[:, t0:t0 + G], v[:, :G, :s1], axis=AX.X, op=ALU.add)
                nc.vector.tensor_reduce(
                    scolN[:, t0:t0 + G], v[:, :G, s1:HW6], axis=AX.X,
                    op=ALU.add)

            while t < T:
                if t + 2 <= T:
                    qp = ps_pair.tile([128, 1024], F32, tag="qp")
                    qv = qp[:].rearrange("p (g c) -> p g c", c=512)
                    for g in range(2):
                        nc.tensor.matmul(qv[:, g, :HW6],
                                         netb[:, (t + g) * D:(t + g + 1) * D],
                                         w6n[:], start=True, stop=True)
                    if pend is None:
                        scr4 = scrp.tile([128, 4 * HW6], F16, tag="scr4")
                        half = scr4[:, :2 * HW6]
                        pend = (scr4, t)
                    else:
                        scr4, t0 = pend
                        half = scr4[:, 2 * HW6:]
                    nc.scalar.activation(
                        half.rearrange("p (g c) -> p g c", c=HW6),
                        qv[:, :, :HW6], AF.Relu)
                    if pend[1] != t:
                        reduce_group(scr4, t0, 4)
                        pend = None
                    t += 2
                else:
                    s_ps = ps_sc.tile([128, HW6], F32, tag="sc")
                    nc.tensor.matmul(s_ps[:], netb[:, t * D:(t + 1) * D],
                                     w6n[:], start=True, stop=True)
                    scr = scrp.tile([128, HW6], F16, tag="scr")
                    nc.vector.scalar_tensor_tensor(
                        scr[:], s_ps[:], 0.0, cfn[:],
                        op0=ALU.max, op1=ALU.mult,
                        accum_out=scolP[:, t:t + 1])
                    t += 1
            if pend is not None:
                reduce_group(pend[0], pend[1], 2)

            return dict(nenb=nenb, T=T, F=F, boff=boff, scolP=scolP,
                        scolN=scolN, sx=sx)

        def phaseB(st):
            """Softmax (unnormalized exp + reciprocal-Z) and attention weights."""
            T, F = st["T"], st["F"]
            scolP, scolN, sx = st["scolP"], st["scolN"], st["sx"]

            # sxs[p,t] = sx[4t + p//32] via selector matmul
            Rm = smallp.tile([128, NB], F32, tag="Rm")
            nc.gpsimd.tensor_tensor(
                Rm[:, :T], sx[:, 0:1].broadcast_to([128, T]), selm[:, :T],
                op=ALU.mult)
            sxs_ps = ps_misc.tile([128, NB], F32, tag="misc")
            nc.tensor.matmul(sxs_ps[:, :T], bselx[:], Rm[:, :T],
                             start=True, stop=True)

            z0 = smallp.tile([128, NB], F32, tag="z0")
            nc.gpsimd.tensor_tensor(z0[:, :T], scolP[:, :T], scolN[:, :T],
                                    op=ALU.subtract)
            z = smallp.tile([128, NB], F32, tag="z")
            nc.vector.tensor_tensor(z[:, :T], z0[:, :T], sxs_ps[:, :T],
                                    op=ALU.add)
            zl = smallp.tile([128, NB], F32, tag="zl")
            nc.vector.scalar_tensor_tensor(zl[:, :T], z[:, :T], 0.2, z[:, :T],
                                           op0=ALU.mult, op1=ALU.max)
            ew = smallp.tile([128, NB], F32, tag="ew")
            nc.scalar.activation(ew[:, :T], zl[:, :T], AF.Exp)

            # Z per node, reciprocal, broadcast back to [p, t]
            zt_ps = ps_misc.tile([128, NB], F32, tag="misc")
            nc.tensor.matmul(zt_ps[:4, :T], psel4[:], ew[:, :T],
                             start=True, stop=True)
            rz4 = smallp.tile([4, NB], F32, tag="rz4")
            nc.vector.reciprocal(rz4[:4, :T], zt_ps[:4, :T])
            rzf_ps = ps_misc.tile([128, NB], F32, tag="misc")
            nc.tensor.matmul(rzf_ps[:, :T], bsel4[:4, :], rz4[:4, :T],
                             start=True, stop=True)
            ewn = smallp.tile([128, NB], F32, tag="ewn")
            nc.vector.tensor_tensor(ewn[:, :T], ew[:, :T], rzf_ps[:, :T],
                                    op=ALU.mult)

            aw = awp.tile([128, 128], F16, tag="aw")
            nc.gpsimd.tensor_tensor(
                aw[:].rearrange("p (t j) -> p t j", j=4)[:, :T, :],
                ewn[:, :T].unsqueeze(2).broadcast_to([128, T, 4]),
                mask4[:].unsqueeze(1).broadcast_to([128, T, 4]),
                op=ALU.mult)
            st["aw"] = aw

        def phaseC(st):
            """Attention apply (agg), output matmuls, relu, store."""
            nenb, aw = st["nenb"], st["aw"]
            T, F, boff = st["T"], st["F"], st["boff"]
            nen_v = nenb[:].rearrange("p (t d) -> p t d", d=D)
            aw_v = aw[:].rearrange("p (t j) -> p t j", j=4)
            agg_ps = ps_agg.tile([128, 128], F32, tag="agg")
            for t in range(T):
                nc.tensor.matmul(agg_ps[:, 4 * t:4 * (t + 1)], nen_v[:, t, :],
                                 aw_v[:, t, :], start=True, stop=True)
            aggt = awp.tile([D, 128], F16, tag="aggt")
            nc.vector.tensor_copy(aggt[:, :F], agg_ps[:, :F])

            fc_ps = ps_fc.tile([128, 2 * O], F32, tag="fc")
            nc.tensor.matmul(fc_ps[:F, 0:O], xt_all[:, boff:boff + F], wfcx[:],
                             start=True, stop=True)
            nc.tensor.matmul(fc_ps[:F, O:2 * O], aggt[:, :F], wfcn[:],
                             start=True, stop=True)
            out_sb = outp.tile([128, 2 * O], F32, tag="out")
            nc.scalar.activation(out_sb[:F, :], fc_ps[:F, :], AF.Relu)
            nc.sync.dma_start(out_d[boff:boff + F, :], out_sb[:F, :])

        prev = None
        for (boff, F) in _blocks(bc):
            st = phaseA(boff, F)
            if prev is not None:
                phaseC(prev)
            phaseB(st)
            prev = st
        phaseC(prev)

    nc.compile()
    _PROG_CACHE[key] = nc
    return nc


def kernel(x, neibs, W_att, W_fcx, W_fcn, a, n_cores=N_CORES):
    x = np.asarray(x, dtype=np.float32)
    neibs = np.asarray(neibs, dtype=np.float32)
    W_att = np.asarray(W_att, dtype=np.float32)
    W_fcx = np.asarray(W_fcx, dtype=np.float32)
    W_fcn = np.asarray(W_fcn, dtype=np.float32)
    a = np.asarray(a, dtype=np.float32)

    B = x.shape[0]
    bc = B // n_cores
    a_x, a_n = a[:H, 0], a[H:, 0]
    w6x_np, split_x = _score_weights(W_att, a_x)
    w6n_np, split_n = _score_weights(W_att, a_n)

    nc = _build_program(bc, split_n, split_x, n_cores)

    def cful(split, rep=1):
        v = np.concatenate([np.ones(split), -np.ones(HW6 - split)])
        v = np.tile(v, rep)
        return np.repeat(v[None, :].astype(np.float16), 128, axis=0)

    p = np.arange(128)
    psel4_np = np.equal.outer(p // 32, np.arange(4)).astype(np.float32)
    bsel4_np = np.equal.outer(np.arange(4), p // 32).astype(np.float32)
    bselx_np = np.equal.outer(p % 4, p // 32).astype(np.float32)
    selm_np = np.equal.outer(p // 4, np.arange(NB)).astype(np.float32)
    mask4_np = np.equal.outer(p // 32, np.arange(4)).astype(np.float16)

    shared = {
        "w6n": w6n_np.astype(np.float16), "w6x": w6x_np.astype(np.float16),
        "cfn": cful(split_n), "cfx": cful(split_x),
        "wfcx": W_fcx.astype(np.float16), "wfcn": W_fcn.astype(np.float16),
        "psel4": psel4_np, "bsel4": bsel4_np, "bselx": bselx_np,
        "selm": selm_np, "mask4": mask4_np,
    }

    rows_c = bc * NB
    tiles_c = rows_c // 128
    in_maps = []
    for c in range(n_cores):
        sl = neibs[c * rows_c:(c + 1) * rows_c]
        neT_np = np.ascontiguousarray(sl.T).astype(np.float16)
        neN_np = np.ascontiguousarray(
            sl.reshape(tiles_c, 128, D).transpose(1, 0, 2).reshape(128, rows_c)
        ).astype(np.float16)
        xT_np = np.ascontiguousarray(x[c * bc:(c + 1) * bc].T).astype(np.float16)
        in_maps.append({
            "neT": neT_np, "neN": neN_np, "xT": xT_np, **shared,
        })
    res = run_bass_kernel_spmd(nc, in_maps, core_ids=list(range(n_cores)),
                               **TRACE_OPTS)
    LAST_RESULT[0] = res
    return np.concatenate([res.results[c]["out"] for c in range(n_cores)], axis=0)


# revision 39
# speedup vs baseline: 1.2872x; 1.1919x over previous
"""AttentionAggregator Trainium2 kernel (8-core SPMD, data-parallel over nodes).

Math (per node b with neighbors n):
  x_att   = lrelu_.01(x @ W_att);  neib_att = lrelu_.01(neibs @ W_att)
  e[b,n]  = lrelu_.2(x_att[b]@a_x + neib_att[b,n]@a_n)
  att     = softmax_n(e)
  agg[b]  = sum_n att[b,n] * neibs[b,n]
  out     = relu([x@W_fcx, agg@W_fcn])

Design (vs a PE-transpose-per-tile formulation):
  - Host pre-builds TWO fp16 layouts of neibs per core: neT (transposed
    [D, rows] — feeds score matmuls directly, no on-chip transposes) and neN
    (rows-grouped-by-128 natural [p, (tile, d)] — feeds the attention-apply
    matmuls).  Both DMA at 8KB contiguous per partition-line per block.
  - x is pre-transposed on host (xT [D, B]) and persists in SBUF: serves as
    lhsT for both the x-score matmul and the x@W_fcx output matmul.
  - Scores: 258-col relu-pair decomposition of a.lrelu(W z) (exact).  Score
    matmuls land in two-bank PSUM pairs; ACT relu's two pairs into a 4-tile
    fp16 buffer; DVE then does ONE grouped tensor_reduce per sign-segment
    covering 4 tiles (no accumulator-register reads).
  - Softmax normalization without cross-partition shuffles: per-node sums,
    reciprocal-broadcast and x-score broadcast are done with tiny constant
    selector matmuls on the PE (psel4/bsel4/bselx); exp is unnormalized and
    1/Z is folded in before the attention-apply matmuls.
  - GPSIMD (no PSUM access) takes the all-SBUF elementwise builds (selector
    products, attention-weight masking) off the DVE.
"""
import warnings
warnings.filterwarnings("ignore")
import numpy as np
from contextlib import ExitStack

import concourse.bass as bass
import concourse.tile as tile
from concourse import bacc, mybir
from concourse.bass_utils import run_bass_kernel_spmd

F32 = mybir.dt.float32
F16 = mybir.dt.float16
AF = mybir.ActivationFunctionType
ALU = mybir.AluOpType
AX = mybir.AxisListType

N_CORES = 8
B_FULL, NB, D, H, O = 20000, 32, 128, 256, 128
HW6 = 2 * H // 2 + 2  # 258 score columns

# Test-harness knobs (ignored by the grading harness, which calls kernel()
# directly): set TRACE_OPTS["trace"]=True to capture an NTFF profile; the
# BassKernelResults of the last run lands in LAST_RESULT[0].
TRACE_OPTS = {}
LAST_RESULT = [None]




def _score_weights(W_att: np.ndarray, a_half: np.ndarray):
    """Build the 258-column relu-pair score weight matrix. Returns (W6, split)."""
    pos = np.where(a_half >= 0)[0]
    neg = np.where(a_half < 0)[0]
    Wabs = W_att * np.abs(a_half)[None, :]
    w_d = (W_att @ a_half).astype(np.float64)
    seg1 = np.concatenate([0.99 * Wabs[:, pos], 0.01 * w_d[:, None]], axis=1)
    seg2 = np.concatenate([0.99 * Wabs[:, neg], -0.01 * w_d[:, None]], axis=1)
    W6 = np.concatenate([seg1, seg2], axis=1).astype(np.float32)
    return W6, seg1.shape[1]


def _blocks(bc):
    out = []
    o = 0
    while o < bc:
        f = min(128, bc - o)
        assert f * NB % 128 == 0
        out.append((o, f))
        o += f
    return out


_PROG_CACHE = {}


def _build_program(bc, split_n, split_x, n_cores=N_CORES):
    key = (bc, split_n, split_x, n_cores)
    if key in _PROG_CACHE:
        return _PROG_CACHE[key]

    nc = bacc.Bacc("TRN2", target_bir_lowering=False, debug=False,
                   num_devices=n_cores)

    R = bc * NB  # neighbor rows per core
    neT_d = nc.dram_tensor("neT", [D, R], F16, kind="ExternalInput").ap()
    neN_d = nc.dram_tensor("neN", [128, R], F16, kind="ExternalInput").ap()
    xT_d = nc.dram_tensor("xT", [D, bc], F16, kind="ExternalInput").ap()
    w6n_d = nc.dram_tensor("w6n", [D, HW6], F16, kind="ExternalInput").ap()
    w6x_d = nc.dram_tensor("w6x", [D, HW6], F16, kind="ExternalInput").ap()
    cfn_d = nc.dram_tensor("cfn", [128, HW6], F16, kind="ExternalInput").ap()
    cfx_d = nc.dram_tensor("cfx", [128, HW6], F16, kind="ExternalInput").ap()
    wfcx_d = nc.dram_tensor("wfcx", [D, O], F16, kind="ExternalInput").ap()
    wfcn_d = nc.dram_tensor("wfcn", [D, O], F16, kind="ExternalInput").ap()
    psel4_d = nc.dram_tensor("psel4", [128, 4], F32, kind="ExternalInput").ap()
    bsel4_d = nc.dram_tensor("bsel4", [4, 128], F32, kind="ExternalInput").ap()
    bselx_d = nc.dram_tensor("bselx", [128, 128], F32, kind="ExternalInput").ap()
    selm_d = nc.dram_tensor("selm", [128, NB], F32, kind="ExternalInput").ap()
    mask4_d = nc.dram_tensor("mask4", [128, 4], F16, kind="ExternalInput").ap()
    out_d = nc.dram_tensor("out", [bc, 2 * O], F32, kind="ExternalOutput").ap()

    with tile.TileContext(nc) as tc, ExitStack() as ctx:
        consts = ctx.enter_context(tc.tile_pool(name="consts", bufs=1))
        netp = ctx.enter_context(tc.tile_pool(name="netp", bufs=3))
        nenp = ctx.enter_context(tc.tile_pool(name="nenp", bufs=3))
        scrp = ctx.enter_context(tc.tile_pool(name="scrp", bufs=4))
        smallp = ctx.enter_context(tc.tile_pool(name="smallp", bufs=3))
        awp = ctx.enter_context(tc.tile_pool(name="awp", bufs=3))
        outp = ctx.enter_context(tc.tile_pool(name="outp", bufs=2))
        ps_sc = ctx.enter_context(tc.tile_pool(name="ps_sc", bufs=1, space="PSUM"))
        ps_pair = ctx.enter_context(tc.tile_pool(name="ps_pair", bufs=2, space="PSUM"))
        ps_misc = ctx.enter_context(tc.tile_pool(name="ps_misc", bufs=1, space="PSUM"))
        ps_agg = ctx.enter_context(tc.tile_pool(name="ps_agg", bufs=1, space="PSUM"))
        ps_fc = ctx.enter_context(tc.tile_pool(name="ps_fc", bufs=1, space="PSUM"))

        w6n = consts.tile([D, HW6], F16)
        w6x = consts.tile([D, HW6], F16)
        cfn = consts.tile([128, HW6], F16)
        cfx = consts.tile([128, HW6], F16)
        wfcx = consts.tile([D, O], F16)
        wfcn = consts.tile([D, O], F16)
        psel4 = consts.tile([128, 4], F32)
        bsel4 = consts.tile([4, 128], F32)
        bselx = consts.tile([128, 128], F32)
        selm = consts.tile([128, NB], F32)
        mask4 = consts.tile([128, 4], F16)
        xt_all = consts.tile([D, bc], F16)
        for t, d in [(w6n, w6n_d), (w6x, w6x_d), (cfn, cfn_d), (cfx, cfx_d),
                     (wfcx, wfcx_d), (wfcn, wfcn_d), (psel4, psel4_d),
                     (bsel4, bsel4_d), (bselx, bselx_d), (selm, selm_d),
                     (mask4, mask4_d), (xt_all, xT_d)]:
            nc.sync.dma_start(t[:], d)

        def phaseA(boff, F):
            """DMA loads, score matmuls + drains, x-side score."""
            T = F * NB // 128
            rbase = boff * NB

            netb = netp.tile([128, 32 * D], F16, tag="net")
            nc.sync.dma_start(netb[:, :T * D], neT_d[:, rbase:rbase + 128 * T])
            nenb = nenp.tile([128, 32 * D], F16, tag="nen")
            nc.sync.dma_start(nenb[:, :T * D], neN_d[:, rbase:rbase + 128 * T])

            scolP = smallp.tile([128, NB], F32, tag="scolP")
            scolN = smallp.tile([128, NB], F32, tag="scolN")
            nc.gpsimd.memset(scolN[:, :T], 0.0)

            # x-side score (drain on DVE)
            xs_ps = ps_sc.tile([128, HW6], F32, tag="sc")
            nc.tensor.matmul(xs_ps[:F, :], xt_all[:, boff:boff + F], w6x[:],
                             start=True, stop=True)
            xscr = scrp.tile([128, HW6], F16, tag="xscr")
            sx = smallp.tile([128, 1], F32, tag="sx")
            nc.vector.scalar_tensor_tensor(
                xscr[:F, :], xs_ps[:F, :], 0.0, cfx[:F, :],
                op0=ALU.max, op1=ALU.mult, accum_out=sx[:F, :])

            # neighbor score tiles: PSUM pairs relu'd by ACT into halves of a
            # 4-tile fp16 buffer; DVE then does one grouped tensor_reduce per
            # sign-segment covering 4 tiles.  Odd tail tile drains direct.
            s1 = split_n
            t = 0
            pend = None

            def reduce_group(scr4, t0, G):
                v = scr4[:].rearrange("p (g c) -> p g c", c=HW6)
                nc.vector.tensor_reduce(
                    scolP[:, t0:t0 + G], v[:, :G, :s1], axis=AX.X, op=ALU.add)
                nc.vector.tensor_reduce(
                    scolN[:, t0:t0 + G], v[:, :G, s1:HW6], axis=AX.X,
                    op=ALU.add)

            while t < T:
                if t + 2 <= T:
                    qp = ps_pair.tile([128, 1024], F32, tag="qp")
                    qv = qp[:].rearrange("p (g c) -> p g c", c=512)
                    for g in range(2):
                        nc.tensor.matmul(qv[:, g, :HW6],
                                         netb[:, (t + g) * D:(t + g + 1) * D],
                                         w6n[:], start=True, stop=True)
                    if pend is None:
                        scr4 = scrp.tile([128, 4 * HW6], F16, tag="scr4")
                        half = scr4[:, :2 * HW6]
                        pend = (scr4, t)
                    else:
                        scr4, t0 = pend
                        half = scr4[:, 2 * HW6:]
                    nc.scalar.activation(
                        half.rearrange("p (g c) -> p g c", c=HW6),
                        qv[:, :, :HW6], AF.Relu)
                    if pend[1] != t:
                        reduce_group(scr4, t0, 4)
                        pend = None
                    t += 2
                else:
                    s_ps = ps_sc.tile([128, HW6], F32, tag="sc")
                    nc.tensor.matmul(s_ps[:], netb[:, t * D:(t + 1) * D],
                                     w6n[:], start=True, stop=True)
                    scr = scrp.tile([128, HW6], F16, tag="scr")
                    nc.vector.scalar_tensor_tensor(
                        scr[:], s_ps[:], 0.0, cfn[:],
                        op0=ALU.max, op1=ALU.mult,
                        accum_out=scolP[:, t:t + 1])
                    t += 1
            if pend is not None:
                reduce_group(pend[0], pend[1], 2)

            return dict(nenb=nenb, T=T, F=F, boff=boff, scolP=scolP,
                        scolN=scolN, sx=sx)

        def phaseB(st):
            """Softmax (unnormalized exp + reciprocal-Z) and attention weights."""
            T, F = st["T"], st["F"]
            scolP, scolN, sx = st["scolP"], st["scolN"], st["sx"]

            # sxs[p,t] = sx[4t + p//32] via selector matmul
            Rm = smallp.tile([128, NB], F32, tag="Rm")
            nc.gpsimd.tensor_tensor(
                Rm[:, :T], sx[:, 0:1].broadcast_to([128, T]), selm[:, :T],
                op=ALU.mult)
            sxs_ps = ps_misc.tile([128, NB], F32, tag="misc")
            nc.tensor.matmul(sxs_ps[:, :T], bselx[:], Rm[:, :T],
                             start=True, stop=True)

            z0 = smallp.tile([128, NB], F32, tag="z0")
            nc.gpsimd.tensor_tensor(z0[:, :T], scolP[:, :T], scolN[:, :T],
                                    op=ALU.subtract)
            z = smallp.tile([128, NB], F32, tag="z")
            nc.vector.tensor_tensor(z[:, :T], z0[:, :T], sxs_ps[:, :T],
                                    op=ALU.add)
            zl = smallp.tile([128, NB], F32, tag="zl")
            nc.vector.scalar_tensor_tensor(zl[:, :T], z[:, :T], 0.2, z[:, :T],
                                           op0=ALU.mult, op1=ALU.max)
            ew = smallp.tile([128, NB], F32, tag="ew")
            nc.scalar.activation(ew[:, :T], zl[:, :T], AF.Exp)

            # Z per node, reciprocal, broadcast back to [p, t]
            zt_ps = ps_misc.tile([128, NB], F32, tag="misc")
            nc.tensor.matmul(zt_ps[:4, :T], psel4[:], ew[:, :T],
                             start=True, stop=True)
            rz4 = smallp.tile([4, NB], F32, tag="rz4")
            nc.vector.reciprocal(rz4[:4, :T], zt_ps[:4, :T])
            rzf_ps = ps_misc.tile([128, NB], F32, tag="misc")
            nc.tensor.matmul(rzf_ps[:, :T], bsel4[:4, :], rz4[:4, :T],
                             start=True, stop=True)
            ewn = smallp.tile([128, NB], F32, tag="ewn")
            nc.vector.tensor_tensor(ewn[:, :T], ew[:, :T], rzf_ps[:, :T],
                                    op=ALU.mult)

            aw = awp.tile([128, 128], F16, tag="aw")
            nc.gpsimd.tensor_tensor(
                aw[:].rearrange("p (t j) -> p t j", j=4)[:, :T, :],
                ewn[:, :T].unsqueeze(2).broadcast_to([128, T, 4]),
                mask4[:].unsqueeze(1).broadcast_to([128, T, 4]),
                op=ALU.mult)
            st["aw"] = aw

        def phaseC(st):
            """Attention apply (agg), output matmuls, relu, store."""
            nenb, aw = st["nenb"], st["aw"]
            T, F, boff = st["T"], st["F"], st["boff"]
            nen_v = nenb[:].rearrange("p (t d) -> p t d", d=D)
            aw_v = aw[:].rearrange("p (t j) -> p t j", j=4)
            agg_ps = ps_agg.tile([128, 128], F32, tag="agg")
            for t in range(T):
                nc.tensor.matmul(agg_ps[:, 4 * t:4 * (t + 1)], nen_v[:, t, :],
                                 aw_v[:, t, :], start=True, stop=True)
            aggt = awp.tile([D, 128], F16, tag="aggt")
            nc.vector.tensor_copy(aggt[:, :F], agg_ps[:, :F])

            fc_ps = ps_fc.tile([128, 2 * O], F32, tag="fc")
            nc.tensor.matmul(fc_ps[:F, 0:O], xt_all[:, boff:boff + F], wfcx[:],
                             start=True, stop=True)
            nc.tensor.matmul(fc_ps[:F, O:2 * O], aggt[:, :F], wfcn[:],
                             start=True, stop=True)
            out_sb = outp.tile([128, 2 * O], F32, tag="out")
            nc.scalar.activation(out_sb[:F, :], fc_ps[:F, :], AF.Relu)
            nc.sync.dma_start(out_d[boff:boff + F, :], out_sb[:F, :])

        prev = None
        for (boff, F) in _blocks(bc):
            st = phaseA(boff, F)
            if prev is not None:
                phaseC(prev)
            phaseB(st)
            prev = st
        phaseC(prev)

    nc.compile()
    _PROG_CACHE[key] = nc
    return nc


def kernel(x, neibs, W_att, W_fcx, W_fcn, a, n_cores=N_CORES):
    x = np.asarray(x, dtype=np.float32)
    neibs = np.asarray(neibs, dtype=np.float32)
    W_att = np.asarray(W_att, dtype=np.float32)
    W_fcx = np.asarray(W_fcx, dtype=np.float32)
    W_fcn = np.asarray(W_fcn, dtype=np.float32)
    a = np.asarray(a, dtype=np.float32)

    B = x.shape[0]
    bc = B // n_cores
    a_x, a_n = a[:H, 0], a[H:, 0]
    w6x_np, split_x = _score_weights(W_att, a_x)
    w6n_np, split_n = _score_weights(W_att, a_n)

    nc = _build_program(bc, split_n, split_x, n_cores)

    def cful(split):
        v = np.concatenate([np.ones(split), -np.ones(HW6 - split)])
        return np.repeat(v[None, :].astype(np.float16), 128, axis=0)

    p = np.arange(128)
    psel4_np = np.equal.outer(p // 32, np.arange(4)).astype(np.float32)
    bsel4_np = np.equal.outer(np.arange(4), p // 32).astype(np.float32)
    bselx_np = np.equal.outer(p % 4, p // 32).astype(np.float32)
    selm_np = np.equal.outer(p // 4, np.arange(NB)).astype(np.float32)
    mask4_np = np.equal.outer(p // 32, np.arange(4)).astype(np.float16)

    shared = {
        "w6n": w6n_np.astype(np.float16), "w6x": w6x_np.astype(np.float16),
        "cfn": cful(split_n), "cfx": cful(split_x),
        "wfcx": W_fcx.astype(np.float16), "wfcn": W_fcn.astype(np.float16),
        "psel4": psel4_np, "bsel4": bsel4_np, "bselx": bselx_np,
        "selm": selm_np, "mask4": mask4_np,
    }

    rows_c = bc * NB
    tiles_c = rows_c // 128
    in_maps = []
    for c in range(n_cores):
        sl = neibs[c * rows_c:(c + 1) * rows_c]
        neT_np = np.ascontiguousarray(sl.T).astype(np.float16)
        neN_np = np.ascontiguousarray(
            sl.reshape(tiles_c, 128, D).transpose(1, 0, 2).reshape(128, rows_c)
        ).astype(np.float16)
        xT_np = np.ascontiguousarray(x[c * bc:(c + 1) * bc].T).astype(np.float16)
        in_maps.append({
            "neT": neT_np, "neN": neN_np, "xT": xT_np, **shared,
        })
    res = run_bass_kernel_spmd(nc, in_maps, core_ids=list(range(n_cores)),
                               **TRACE_OPTS)
    LAST_RESULT[0] = res
    return np.concatenate([res.results[c]["out"] for c in range(n_cores)], axis=0)


# revision 42
# speedup vs baseline: 1.4803x; 1.1500x over previous
"""AttentionAggregator Trainium2 kernel (8-core SPMD, data-parallel over nodes).

Math (per node b with neighbors n):
  x_att   = lrelu_.01(x @ W_att);  neib_att = lrelu_.01(neibs @ W_att)
  e[b,n]  = lrelu_.2(x_att[b]@a_x + neib_att[b,n]@a_n)
  att     = softmax_n(e)
  agg[b]  = sum_n att[b,n] * neibs[b,n]
  out     = relu([x@W_fcx, agg@W_fcn])

Design (vs a PE-transpose-per-tile formulation):
  - Host pre-builds TWO fp16 layouts of neibs per core: neT (transposed
    [D, rows] — feeds score matmuls directly, no on-chip transposes) and neN
    (rows-grouped-by-128 natural [p, (tile, d)] — feeds the attention-apply
    matmuls).  Both DMA at 8KB contiguous per partition-line per block.
  - x is pre-transposed on host (xT [D, B]) and persists in SBUF: serves as
    lhsT for both the x-score matmul and the x@W_fcx output matmul.
  - Scores: 258-col relu-pair decomposition of a.lrelu(W z) (exact).  Score
    matmuls land in two-bank PSUM pairs; ACT relu's two pairs into a 4-tile
    fp16 buffer; DVE then does ONE grouped tensor_reduce per sign-segment
    covering 4 tiles (no accumulator-register reads).
  - Softmax normalization without cross-partition shuffles: per-node sums,
    reciprocal-broadcast and x-score broadcast are done with tiny constant
    selector matmuls on the PE (psel4/bsel4/bselx); exp is unnormalized and
    1/Z is folded in before the attention-apply matmuls.
  - GPSIMD (no PSUM access) takes the all-SBUF elementwise builds (selector
    products, attention-weight masking) off the DVE.
"""
import warnings
warnings.filterwarnings("ignore")
import numpy as np
from contextlib import ExitStack

import concourse.bass as bass
import concourse.tile as tile
from concourse import bacc, mybir
from concourse.bass_utils import run_bass_kernel_spmd

F32 = mybir.dt.float32
F16 = mybir.dt.float16
AF = mybir.ActivationFunctionType
ALU = mybir.AluOpType
AX = mybir.AxisListType

N_CORES = 8
B_FULL, NB, D, H, O = 20000, 32, 128, 256, 128
HW6 = 2 * H // 2 + 2  # 258 score columns

# Test-harness knobs (ignored by the grading harness, which calls kernel()
# directly): set TRACE_OPTS["trace"]=True to capture an NTFF profile; the
# BassKernelResults of the last run lands in LAST_RESULT[0].
TRACE_OPTS = {}
LAST_RESULT = [None]




def _score_weights(W_att: np.ndarray, a_half: np.ndarray):
    """Build the 258-column relu-pair score weight matrix. Returns (W6, split)."""
    pos = np.where(a_half >= 0)[0]
    neg = np.where(a_half < 0)[0]
    Wabs = W_att * np.abs(a_half)[None, :]
    w_d = (W_att @ a_half).astype(np.float64)
    seg1 = np.concatenate([0.99 * Wabs[:, pos], 0.01 * w_d[:, None]], axis=1)
    seg2 = np.concatenate([0.99 * Wabs[:, neg], -0.01 * w_d[:, None]], axis=1)
    W6 = np.concatenate([seg1, seg2], axis=1).astype(np.float32)
    return W6, seg1.shape[1]


def _blocks(bc):
    out = []
    o = 0
    while o < bc:
        f = min(128, bc - o)
        assert f * NB % 128 == 0
        out.append((o, f))
        o += f
    return out


_PROG_CACHE = {}


def _build_program(bc, split_n, split_x, n_cores=N_CORES):
    key = (bc, split_n, split_x, n_cores)
    if key in _PROG_CACHE:
        return _PROG_CACHE[key]

    nc = bacc.Bacc("TRN2", target_bir_lowering=False, debug=False,
                   num_devices=n_cores)

    R = bc * NB  # neighbor rows per core
    neT_d = nc.dram_tensor("neT", [D, R], F16, kind="ExternalInput").ap()
    neN_d = nc.dram_tensor("neN", [128, R], F16, kind="ExternalInput").ap()
    xT_d = nc.dram_tensor("xT", [D, bc], F16, kind="ExternalInput").ap()
    w6n_d = nc.dram_tensor("w6n", [D, HW6], F16, kind="ExternalInput").ap()
    w6x_d = nc.dram_tensor("w6x", [D, HW6], F16, kind="ExternalInput").ap()
    cfn_d = nc.dram_tensor("cfn", [128, HW6], F16, kind="ExternalInput").ap()
    cfx_d = nc.dram_tensor("cfx", [128, HW6], F16, kind="ExternalInput").ap()
    wfcx_d = nc.dram_tensor("wfcx", [D, O], F16, kind="ExternalInput").ap()
    wfcn_d = nc.dram_tensor("wfcn", [D, O], F16, kind="ExternalInput").ap()
    psel4_d = nc.dram_tensor("psel4", [128, 4], F32, kind="ExternalInput").ap()
    bsel4_d = nc.dram_tensor("bsel4", [4, 128], F32, kind="ExternalInput").ap()
    bselx_d = nc.dram_tensor("bselx", [128, 128], F32, kind="ExternalInput").ap()
    selm_d = nc.dram_tensor("selm", [128, NB], F32, kind="ExternalInput").ap()
    mask4_d = nc.dram_tensor("mask4", [128, 4], F16, kind="ExternalInput").ap()
    out_d = nc.dram_tensor("out", [bc, 2 * O], F32, kind="ExternalOutput").ap()

    with tile.TileContext(nc) as tc, ExitStack() as ctx:
        consts = ctx.enter_context(tc.tile_pool(name="consts", bufs=1))
        netp = ctx.enter_context(tc.tile_pool(name="netp", bufs=3))
        nenp = ctx.enter_context(tc.tile_pool(name="nenp", bufs=3))
        scrp = ctx.enter_context(tc.tile_pool(name="scrp", bufs=4))
        smallp = ctx.enter_context(tc.tile_pool(name="smallp", bufs=3))
        awp = ctx.enter_context(tc.tile_pool(name="awp", bufs=3))
        outp = ctx.enter_context(tc.tile_pool(name="outp", bufs=2))
        ps_pair = ctx.enter_context(tc.tile_pool(name="ps_pair", bufs=3, space="PSUM"))
        ps_misc = ctx.enter_context(tc.tile_pool(name="ps_misc", bufs=1, space="PSUM"))
        ps_mix = ctx.enter_context(tc.tile_pool(name="ps_mix", bufs=1, space="PSUM"))

        w6n = consts.tile([D, HW6], F16)
        w6x = consts.tile([D, HW6], F16)
        cfn = consts.tile([128, HW6], F16)
        cfx = consts.tile([128, HW6], F16)
        wfcx = consts.tile([D, O], F16)
        wfcn = consts.tile([D, O], F16)
        psel4 = consts.tile([128, 4], F32)
        bsel4 = consts.tile([4, 128], F32)
        bselx = consts.tile([128, 128], F32)
        selm = consts.tile([128, NB], F32)
        mask4 = consts.tile([128, 4], F16)
        xt_all = consts.tile([D, bc], F16)
        for t, d in [(w6n, w6n_d), (w6x, w6x_d), (cfn, cfn_d), (cfx, cfx_d),
                     (wfcx, wfcx_d), (wfcn, wfcn_d), (psel4, psel4_d),
                     (bsel4, bsel4_d), (bselx, bselx_d), (selm, selm_d),
                     (mask4, mask4_d), (xt_all, xT_d)]:
            nc.sync.dma_start(t[:], d)

        def phaseA(boff, F):
            """DMA loads, score matmuls + drains, x-side score."""
            T = F * NB // 128
            rbase = boff * NB

            netb = netp.tile([128, 32 * D], F16, tag="net")
            nc.sync.dma_start(netb[:, :T * D], neT_d[:, rbase:rbase + 128 * T])
            nenb = nenp.tile([128, 32 * D], F16, tag="nen")
            nc.sync.dma_start(nenb[:, :T * D], neN_d[:, rbase:rbase + 128 * T])

            scolP = smallp.tile([128, NB], F32, tag="scolP")
            scolN = smallp.tile([128, NB], F32, tag="scolN")
            nc.gpsimd.memset(scolN[:, :T], 0.0)

            # x-side score (drain on DVE)
            xs_ps = ps_mix.tile([128, HW6], F32, tag="mix")
            nc.tensor.matmul(xs_ps[:F, :], xt_all[:, boff:boff + F], w6x[:],
                             start=True, stop=True)
            xscr = scrp.tile([128, HW6], F16, tag="xscr")
            sx = smallp.tile([128, 1], F32, tag="sx")
            nc.vector.scalar_tensor_tensor(
                xscr[:F, :], xs_ps[:F, :], 0.0, cfx[:F, :],
                op0=ALU.max, op1=ALU.mult, accum_out=sx[:F, :])

            # neighbor score tiles: PSUM pairs relu'd by ACT into halves of a
            # 4-tile fp16 buffer; DVE then does one grouped tensor_reduce per
            # sign-segment covering 4 tiles.  Odd tail tile drains direct.
            s1 = split_n
            t = 0
            pend = None

            def reduce_group(scr4, t0, G):
                v = scr4[:].rearrange("p (g c) -> p g c", c=HW6)
                nc.vector.tensor_reduce(
                    scolP[:, t0:t0 + G], v[:, :G, :s1], axis=AX.X, op=ALU.add)
                nc.vector.tensor_reduce(
                    scolN[:, t0:t0 + G], v[:, :G, s1:HW6], axis=AX.X,
                    op=ALU.add)

            while t < T:
                if t + 2 <= T:
                    qp = ps_pair.tile([128, 1024], F32, tag="qp")
                    qv = qp[:].rearrange("p (g c) -> p g c", c=512)
                    for g in range(2):
                        nc.tensor.matmul(qv[:, g, :HW6],
                                         netb[:, (t + g) * D:(t + g + 1) * D],
                                         w6n[:], start=True, stop=True)
                    if pend is None:
                        scr4 = scrp.tile([128, 4 * HW6], F16, tag="scr4")
                        half = scr4[:, :2 * HW6]
                        pend = (scr4, t)
                    else:
                        scr4, t0 = pend
                        half = scr4[:, 2 * HW6:]
                    nc.scalar.activation(
                        half.rearrange("p (g c) -> p g c", c=HW6),
                        qv[:, :, :HW6], AF.Relu)
                    if pend[1] != t:
                        reduce_group(scr4, t0, 4)
                        pend = None
                    t += 2
                else:
                    s_pt = ps_pair.tile([128, 1024], F32, tag="qp", name="s_pt")
                    s_ps = s_pt[:, :HW6]
                    nc.tensor.matmul(s_ps[:], netb[:, t * D:(t + 1) * D],
                                     w6n[:], start=True, stop=True)
                    scr = scrp.tile([128, HW6], F16, tag="scr")
                    nc.vector.scalar_tensor_tensor(
                        scr[:], s_ps[:], 0.0, cfn[:],
                        op0=ALU.max, op1=ALU.mult,
                        accum_out=scolP[:, t:t + 1])
                    t += 1
            if pend is not None:
                reduce_group(pend[0], pend[1], 2)

            return dict(nenb=nenb, T=T, F=F, boff=boff, scolP=scolP,
                        scolN=scolN, sx=sx)

        def phaseB(st):
            """Softmax (unnormalized exp + reciprocal-Z) and attention weights."""
            T, F = st["T"], st["F"]
            scolP, scolN, sx = st["scolP"], st["scolN"], st["sx"]

            # sxs[p,t] = sx[4t + p//32] via selector matmul
            Rm = smallp.tile([128, NB], F32, tag="Rm")
            nc.gpsimd.tensor_tensor(
                Rm[:, :T], sx[:, 0:1].broadcast_to([128, T]), selm[:, :T],
                op=ALU.mult)
            sxs_ps = ps_misc.tile([128, NB], F32, tag="misc")
            nc.tensor.matmul(sxs_ps[:, :T], bselx[:], Rm[:, :T],
                             start=True, stop=True)

            z0 = smallp.tile([128, NB], F32, tag="z0")
            nc.gpsimd.tensor_tensor(z0[:, :T], scolP[:, :T], scolN[:, :T],
                                    op=ALU.subtract)
            z = smallp.tile([128, NB], F32, tag="z")
            nc.vector.tensor_tensor(z[:, :T], z0[:, :T], sxs_ps[:, :T],
                                    op=ALU.add)
            zl = smallp.tile([128, NB], F32, tag="zl")
            nc.vector.scalar_tensor_tensor(zl[:, :T], z[:, :T], 0.2, z[:, :T],
                                           op0=ALU.mult, op1=ALU.max)
            ew = smallp.tile([128, NB], F32, tag="ew")
            nc.scalar.activation(ew[:, :T], zl[:, :T], AF.Exp)

            # Z per node, reciprocal, broadcast back to [p, t]
            zt_ps = ps_misc.tile([128, NB], F32, tag="misc")
            nc.tensor.matmul(zt_ps[:4, :T], psel4[:], ew[:, :T],
                             start=True, stop=True)
            rz4 = smallp.tile([4, NB], F32, tag="rz4")
            nc.vector.reciprocal(rz4[:4, :T], zt_ps[:4, :T])
            rzf_ps = ps_misc.tile([128, NB], F32, tag="misc")
            nc.tensor.matmul(rzf_ps[:, :T], bsel4[:4, :], rz4[:4, :T],
                             start=True, stop=True)
            ewn = smallp.tile([128, NB], F32, tag="ewn")
            nc.vector.tensor_tensor(ewn[:, :T], ew[:, :T], rzf_ps[:, :T],
                                    op=ALU.mult)

            aw = awp.tile([128, 128], F16, tag="aw")
            nc.gpsimd.tensor_tensor(
                aw[:].rearrange("p (t j) -> p t j", j=4)[:, :T, :],
                ewn[:, :T].unsqueeze(2).broadcast_to([128, T, 4]),
                mask4[:].unsqueeze(1).broadcast_to([128, T, 4]),
                op=ALU.mult)
            st["aw"] = aw

        def phaseC(st):
            """Attention apply (agg), output matmuls, relu, store."""
            nenb, aw = st["nenb"], st["aw"]
            T, F, boff = st["T"], st["F"], st["boff"]
            nen_v = nenb[:].rearrange("p (t d) -> p t d", d=D)
            aw_v = aw[:].rearrange("p (t j) -> p t j", j=4)
            agg_ps = ps_mix.tile([128, HW6], F32, tag="mix")
            for t in range(T):
                nc.tensor.matmul(agg_ps[:, 4 * t:4 * (t + 1)], nen_v[:, t, :],
                                 aw_v[:, t, :], start=True, stop=True)
            aggt = awp.tile([D, 128], F16, tag="aggt")
            nc.vector.tensor_copy(aggt[:, :F], agg_ps[:, :F])

            fc_ps = ps_mix.tile([128, HW6], F32, tag="mix")
            nc.tensor.matmul(fc_ps[:F, 0:O], xt_all[:, boff:boff + F], wfcx[:],
                             start=True, stop=True)
            nc.tensor.matmul(fc_ps[:F, O:2 * O], aggt[:, :F], wfcn[:],
                             start=True, stop=True)
            out_sb = outp.tile([128, 2 * O], F32, tag="out")
            nc.scalar.activation(out_sb[:F, :], fc_ps[:F, :2 * O], AF.Relu)
            nc.sync.dma_start(out_d[boff:boff + F, :], out_sb[:F, :])

        prev = None
        for (boff, F) in _blocks(bc):
            st = phaseA(boff, F)
            if prev is not None:
                phaseC(prev)
            phaseB(st)
            prev = st
        phaseC(prev)

    nc.compile()
    _PROG_CACHE[key] = nc
    return nc


def kernel(x, neibs, W_att, W_fcx, W_fcn, a, n_cores=N_CORES):
    x = np.asarray(x, dtype=np.float32)
    neibs = np.asarray(neibs, dtype=np.float32)
    W_att = np.asarray(W_att, dtype=np.float32)
    W_fcx = np.asarray(W_fcx, dtype=np.float32)
    W_fcn = np.asarray(W_fcn, dtype=np.float32)
    a = np.asarray(a, dtype=np.float32)

    B = x.shape[0]
    bc = B // n_cores
    a_x, a_n = a[:H, 0], a[H:, 0]
    w6x_np, split_x = _score_weights(W_att, a_x)
    w6n_np, split_n = _score_weights(W_att, a_n)

    nc = _build_program(bc, split_n, split_x, n_cores)

    def cful(split):
        v = np.concatenate([np.ones(split), -np.ones(HW6 - split)])
        return np.repeat(v[None, :].astype(np.float16), 128, axis=0)

    p = np.arange(128)
    psel4_np = np.equal.outer(p // 32, np.arange(4)).astype(np.float32)
    bsel4_np = np.equal.outer(np.arange(4), p // 32).astype(np.float32)
    bselx_np = np.equal.outer(p % 4, p // 32).astype(np.float32)
    selm_np = np.equal.outer(p // 4, np.arange(NB)).astype(np.float32)
    mask4_np = np.equal.outer(p // 32, np.arange(4)).astype(np.float16)

    shared = {
        "w6n": w6n_np.astype(np.float16), "w6x": w6x_np.astype(np.float16),
        "cfn": cful(split_n), "cfx": cful(split_x),
        "wfcx": W_fcx.astype(np.float16), "wfcn": W_fcn.astype(np.float16),
        "psel4": psel4_np, "bsel4": bsel4_np, "bselx": bselx_np,
        "selm": selm_np, "mask4": mask4_np,
    }

    rows_c = bc * NB
    tiles_c = rows_c // 128
    in_maps = []
    for c in range(n_cores):
        sl = neibs[c * rows_c:(c + 1) * rows_c]
        neT_np = np.ascontiguousarray(sl.T).astype(np.float16)
        neN_np = np.ascontiguousarray(
            sl.reshape(tiles_c, 128, D).transpose(1, 0, 2).reshape(128, rows_c)
        ).astype(np.float16)
        xT_np = np.ascontiguousarray(x[c * bc:(c + 1) * bc].T).astype(np.float16)
        in_maps.append({
            "neT": neT_np, "neN": neN_np, "xT": xT_np, **shared,
        })
    res = run_bass_kernel_spmd(nc, in_maps, core_ids=list(range(n_cores)),
                               **TRACE_OPTS)
    LAST_RESULT[0] = res
    return np.concatenate([res.results[c]["out"] for c in range(n_cores)], axis=0)


# revision 43
# speedup vs baseline: 1.4811x; 1.0006x over previous
"""AttentionAggregator Trainium2 kernel (8-core SPMD, data-parallel over nodes).

Math (per node b with neighbors n):
  x_att   = lrelu_.01(x @ W_att);  neib_att = lrelu_.01(neibs @ W_att)
  e[b,n]  = lrelu_.2(x_att[b]@a_x + neib_att[b,n]@a_n)
  att     = softmax_n(e)
  agg[b]  = sum_n att[b,n] * neibs[b,n]
  out     = relu([x@W_fcx, agg@W_fcn])

Design (vs a PE-transpose-per-tile formulation):
  - Host pre-builds TWO fp16 layouts of neibs per core: neT (transposed
    [D, rows] — feeds score matmuls directly, no on-chip transposes) and neN
    (rows-grouped-by-128 natural [p, (tile, d)] — feeds the attention-apply
    matmuls).  Both DMA at 8KB contiguous per partition-line per block.
  - x is pre-transposed on host (xT [D, B]) and persists in SBUF: serves as
    lhsT for both the x-score matmul and the x@W_fcx output matmul.
  - Scores: 258-col relu-pair decomposition of a.lrelu(W z) (exact).  Score
    matmuls land in two-bank PSUM pairs; ACT relu's two pairs into a 4-tile
    fp16 buffer; DVE then does ONE grouped tensor_reduce per sign-segment
    covering 4 tiles (no accumulator-register reads).
  - Softmax normalization without cross-partition shuffles: per-node sums,
    reciprocal-broadcast and x-score broadcast are done with tiny constant
    selector matmuls on the PE (psel4/bsel4/bselx); exp is unnormalized and
    1/Z is folded in before the attention-apply matmuls.
  - GPSIMD (no PSUM access) takes the all-SBUF elementwise builds (selector
    products, attention-weight masking) off the DVE.
"""
import warnings
warnings.filterwarnings("ignore")
import numpy as np
from contextlib import ExitStack

import concourse.bass as bass
import concourse.tile as tile
from concourse import bacc, mybir
from concourse.bass_utils import run_bass_kernel_spmd

F32 = mybir.dt.float32
F16 = mybir.dt.float16
AF = mybir.ActivationFunctionType
ALU = mybir.AluOpType
AX = mybir.AxisListType

N_CORES = 8
B_FULL, NB, D, H, O = 20000, 32, 128, 256, 128
HW6 = 2 * H // 2 + 2  # 258 score columns

# Test-harness knobs (ignored by the grading harness, which calls kernel()
# directly): set TRACE_OPTS["trace"]=True to capture an NTFF profile; the
# BassKernelResults of the last run lands in LAST_RESULT[0].
TRACE_OPTS = {}
LAST_RESULT = [None]




def _score_weights(W_att: np.ndarray, a_half: np.ndarray):
    """Build the 258-column relu-pair score weight matrix. Returns (W6, split)."""
    pos = np.where(a_half >= 0)[0]
    neg = np.where(a_half < 0)[0]
    Wabs = W_att * np.abs(a_half)[None, :]
    w_d = (W_att @ a_half).astype(np.float64)
    seg1 = np.concatenate([0.99 * Wabs[:, pos], 0.01 * w_d[:, None]], axis=1)
    seg2 = np.concatenate([0.99 * Wabs[:, neg], -0.01 * w_d[:, None]], axis=1)
    W6 = np.concatenate([seg1, seg2], axis=1).astype(np.float32)
    return W6, seg1.shape[1]


def _blocks(bc):
    out = []
    o = 0
    while o < bc:
        f = min(128, bc - o)
        assert f * NB % 128 == 0
        out.append((o, f))
        o += f
    return out


_PROG_CACHE = {}


def _build_program(bc, split_n, split_x, n_cores=N_CORES):
    key = (bc, split_n, split_x, n_cores)
    if key in _PROG_CACHE:
        return _PROG_CACHE[key]

    nc = bacc.Bacc("TRN2", target_bir_lowering=False, debug=False,
                   num_devices=n_cores)

    R = bc * NB  # neighbor rows per core
    neT_d = nc.dram_tensor("neT", [D, R], F16, kind="ExternalInput").ap()
    neN_d = nc.dram_tensor("neN", [128, R], F16, kind="ExternalInput").ap()
    xT_d = nc.dram_tensor("xT", [D, bc], F16, kind="ExternalInput").ap()
    w6n_d = nc.dram_tensor("w6n", [D, HW6], F16, kind="ExternalInput").ap()
    w6x_d = nc.dram_tensor("w6x", [D, HW6], F16, kind="ExternalInput").ap()
    cfn_d = nc.dram_tensor("cfn", [128, HW6], F16, kind="ExternalInput").ap()
    cfx_d = nc.dram_tensor("cfx", [128, HW6], F16, kind="ExternalInput").ap()
    wfcx_d = nc.dram_tensor("wfcx", [D, O], F16, kind="ExternalInput").ap()
    wfcn_d = nc.dram_tensor("wfcn", [D, O], F16, kind="ExternalInput").ap()
    psel4_d = nc.dram_tensor("psel4", [128, 4], F32, kind="ExternalInput").ap()
    bsel4_d = nc.dram_tensor("bsel4", [4, 128], F32, kind="ExternalInput").ap()
    bselx_d = nc.dram_tensor("bselx", [128, 128], F32, kind="ExternalInput").ap()
    selm_d = nc.dram_tensor("selm", [128, NB], F32, kind="ExternalInput").ap()
    mask4_d = nc.dram_tensor("mask4", [128, 4], F16, kind="ExternalInput").ap()
    out_d = nc.dram_tensor("out", [bc, 2 * O], F32, kind="ExternalOutput").ap()

    with tile.TileContext(nc) as tc, ExitStack() as ctx:
        consts = ctx.enter_context(tc.tile_pool(name="consts", bufs=1))
        netp = ctx.enter_context(tc.tile_pool(name="netp", bufs=3))
        nenp = ctx.enter_context(tc.tile_pool(name="nenp", bufs=3))
        scrp = ctx.enter_context(tc.tile_pool(name="scrp", bufs=4))
        smallp = ctx.enter_context(tc.tile_pool(name="smallp", bufs=3))
        awp = ctx.enter_context(tc.tile_pool(name="awp", bufs=3))
        outp = ctx.enter_context(tc.tile_pool(name="outp", bufs=2))
        ps_pair = ctx.enter_context(tc.tile_pool(name="ps_pair", bufs=3, space="PSUM"))
        ps_misc = ctx.enter_context(tc.tile_pool(name="ps_misc", bufs=1, space="PSUM"))
        ps_mix = ctx.enter_context(tc.tile_pool(name="ps_mix", bufs=1, space="PSUM"))

        w6n = consts.tile([D, HW6], F16)
        w6x = consts.tile([D, HW6], F16)
        cfn = consts.tile([128, HW6], F16)
        cfx = consts.tile([128, HW6], F16)
        wfcx = consts.tile([D, O], F16)
        wfcn = consts.tile([D, O], F16)
        psel4 = consts.tile([128, 4], F32)
        bsel4 = consts.tile([4, 128], F32)
        bselx = consts.tile([128, 128], F32)
        selm = consts.tile([128, NB], F32)
        mask4 = consts.tile([128, 4], F16)
        xt_all = consts.tile([D, bc], F16)
        for t, d in [(w6n, w6n_d), (w6x, w6x_d), (cfn, cfn_d), (cfx, cfx_d),
                     (wfcx, wfcx_d), (wfcn, wfcn_d), (psel4, psel4_d),
                     (bsel4, bsel4_d), (bselx, bselx_d), (selm, selm_d),
                     (mask4, mask4_d), (xt_all, xT_d)]:
            nc.sync.dma_start(t[:], d)

        def phaseA(boff, F):
            """DMA loads, score matmuls + drains, x-side score."""
            T = F * NB // 128
            rbase = boff * NB

            netb = netp.tile([128, 32 * D], F16, tag="net")
            nc.sync.dma_start(netb[:, :T * D], neT_d[:, rbase:rbase + 128 * T])
            nenb = nenp.tile([128, 32 * D], F16, tag="nen")
            nc.sync.dma_start(nenb[:, :T * D], neN_d[:, rbase:rbase + 128 * T])

            scolP = smallp.tile([128, NB], F32, tag="scolP")
            scolN = smallp.tile([128, NB], F32, tag="scolN")
            nc.gpsimd.memset(scolN[:, :T], 0.0)

            # x-side score (drain on DVE)
            xs_ps = ps_mix.tile([128, HW6], F32, tag="mix")
            nc.tensor.matmul(xs_ps[:F, :], xt_all[:, boff:boff + F], w6x[:],
                             start=True, stop=True)
            xscr = scrp.tile([128, HW6], F16, tag="xscr")
            sx = smallp.tile([128, 1], F32, tag="sx")
            nc.vector.scalar_tensor_tensor(
                xscr[:F, :], xs_ps[:F, :], 0.0, cfx[:F, :],
                op0=ALU.max, op1=ALU.mult, accum_out=sx[:F, :])

            # neighbor score tiles: PSUM pairs relu'd by ACT into halves of a
            # 4-tile fp16 buffer; DVE then does one grouped tensor_reduce per
            # sign-segment covering 4 tiles.  Odd tail tile drains direct.
            s1 = split_n
            t = 0
            pend = None

            def reduce_group(scr8, t0, G):
                v = scr8[:].rearrange("p (g c) -> p g c", c=HW6)
                nc.vector.tensor_reduce(
                    scolP[:, t0:t0 + G], v[:, :G, :s1], axis=AX.X, op=ALU.add)
                nc.vector.tensor_reduce(
                    scolN[:, t0:t0 + G], v[:, :G, s1:HW6], axis=AX.X,
                    op=ALU.add)

            while t < T:
                if t + 2 <= T:
                    qp = ps_pair.tile([128, 1024], F32, tag="qp")
                    qv = qp[:].rearrange("p (g c) -> p g c", c=512)
                    for g in range(2):
                        nc.tensor.matmul(qv[:, g, :HW6],
                                         netb[:, (t + g) * D:(t + g + 1) * D],
                                         w6n[:], start=True, stop=True)
                    if pend is None:
                        scr8 = scrp.tile([128, 8 * HW6], F16, tag="scr8")
                        k, t0 = 0, t
                    else:
                        scr8, t0, k = pend
                    nc.scalar.activation(
                        scr8[:, k * 2 * HW6:(k + 1) * 2 * HW6].rearrange(
                            "p (g c) -> p g c", c=HW6),
                        qv[:, :, :HW6], AF.Relu)
                    k += 1
                    if k == 4:
                        reduce_group(scr8, t0, 8)
                        pend = None
                    else:
                        pend = (scr8, t0, k)
                    t += 2
                else:
                    s_pt = ps_pair.tile([128, 1024], F32, tag="qp", name="s_pt")
                    s_ps = s_pt[:, :HW6]
                    nc.tensor.matmul(s_ps[:], netb[:, t * D:(t + 1) * D],
                                     w6n[:], start=True, stop=True)
                    scr = scrp.tile([128, HW6], F16, tag="scr")
                    nc.vector.scalar_tensor_tensor(
                        scr[:], s_ps[:], 0.0, cfn[:],
                        op0=ALU.max, op1=ALU.mult,
                        accum_out=scolP[:, t:t + 1])
                    t += 1
            if pend is not None:
                reduce_group(pend[0], pend[1], 2 * pend[2])

            return dict(nenb=nenb, T=T, F=F, boff=boff, scolP=scolP,
                        scolN=scolN, sx=sx)

        def phaseB(st):
            """Softmax (unnormalized exp + reciprocal-Z) and attention weights."""
            T, F = st["T"], st["F"]
            scolP, scolN, sx = st["scolP"], st["scolN"], st["sx"]

            # sxs[p,t] = sx[4t + p//32] via selector matmul
            Rm = smallp.tile([128, NB], F32, tag="Rm")
            nc.gpsimd.tensor_tensor(
                Rm[:, :T], sx[:, 0:1].broadcast_to([128, T]), selm[:, :T],
                op=ALU.mult)
            sxs_ps = ps_misc.tile([128, NB], F32, tag="misc")
            nc.tensor.matmul(sxs_ps[:, :T], bselx[:], Rm[:, :T],
                             start=True, stop=True)

            z0 = smallp.tile([128, NB], F32, tag="z0")
            nc.gpsimd.tensor_tensor(z0[:, :T], scolP[:, :T], scolN[:, :T],
                                    op=ALU.subtract)
            z = smallp.tile([128, NB], F32, tag="z")
            nc.vector.tensor_tensor(z[:, :T], z0[:, :T], sxs_ps[:, :T],
                                    op=ALU.add)
            zl = smallp.tile([128, NB], F32, tag="zl")
            nc.vector.scalar_tensor_tensor(zl[:, :T], z[:, :T], 0.2, z[:, :T],
                                           op0=ALU.mult, op1=ALU.max)
            ew = smallp.tile([128, NB], F32, tag="ew")
            nc.scalar.activation(ew[:, :T], zl[:, :T], AF.Exp)

            # Z per node, reciprocal, broadcast back to [p, t]
            zt_ps = ps_misc.tile([128, NB], F32, tag="misc")
            nc.tensor.matmul(zt_ps[:4, :T], psel4[:], ew[:, :T],
                             start=True, stop=True)
            rz4 = smallp.tile([4, NB], F32, tag="rz4")
            nc.vector.reciprocal(rz4[:4, :T], zt_ps[:4, :T])
            rzf_ps = ps_misc.tile([128, NB], F32, tag="misc")
            nc.tensor.matmul(rzf_ps[:, :T], bsel4[:4, :], rz4[:4, :T],
                             start=True, stop=True)
            ewn = smallp.tile([128, NB], F32, tag="ewn")
            nc.vector.tensor_tensor(ewn[:, :T], ew[:, :T], rzf_ps[:, :T],
                                    op=ALU.mult)

            aw = awp.tile([128, 128], F16, tag="aw")
            nc.gpsimd.tensor_tensor(
                aw[:].rearrange("p (t j) -> p t j", j=4)[:, :T, :],
                ewn[:, :T].unsqueeze(2).broadcast_to([128, T, 4]),
                mask4[:].unsqueeze(1).broadcast_to([128, T, 4]),
                op=ALU.mult)
            st["aw"] = aw

        def phaseC(st):
            """Attention apply (agg), output matmuls, relu, store."""
            nenb, aw = st["nenb"], st["aw"]
            T, F, boff = st["T"], st["F"], st["boff"]
            nen_v = nenb[:].rearrange("p (t d) -> p t d", d=D)
            aw_v = aw[:].rearrange("p (t j) -> p t j", j=4)
            agg_ps = ps_mix.tile([128, HW6], F32, tag="mix")
            for t in range(T):
                nc.tensor.matmul(agg_ps[:, 4 * t:4 * (t + 1)], nen_v[:, t, :],
                                 aw_v[:, t, :], start=True, stop=True)
            aggt = awp.tile([D, 128], F16, tag="aggt")
            nc.scalar.copy(aggt[:, :F], agg_ps[:, :F])

            fc_ps = ps_mix.tile([128, HW6], F32, tag="mix")
            nc.tensor.matmul(fc_ps[:F, 0:O], xt_all[:, boff:boff + F], wfcx[:],
                             start=True, stop=True)
            nc.tensor.matmul(fc_ps[:F, O:2 * O], aggt[:, :F], wfcn[:],
                             start=True, stop=True)
            out_sb = outp.tile([128, 2 * O], F32, tag="out")
            nc.scalar.activation(out_sb[:F, :], fc_ps[:F, :2 * O], AF.Relu)
            nc.sync.dma_start(out_d[boff:boff + F, :], out_sb[:F, :])

        prev = None
        for (boff, F) in _blocks(bc):
            st = phaseA(boff, F)
            if prev is not None:
                phaseC(prev)
            phaseB(st)
            prev = st
        phaseC(prev)

    nc.compile()
    _PROG_CACHE[key] = nc
    return nc


def kernel(x, neibs, W_att, W_fcx, W_fcn, a, n_cores=N_CORES):
    x = np.asarray(x, dtype=np.float32)
    neibs = np.asarray(neibs, dtype=np.float32)
    W_att = np.asarray(W_att, dtype=np.float32)
    W_fcx = np.asarray(W_fcx, dtype=np.float32)
    W_fcn = np.asarray(W_fcn, dtype=np.float32)
    a = np.asarray(a, dtype=np.float32)

    B = x.shape[0]
    bc = B // n_cores
    a_x, a_n = a[:H, 0], a[H:, 0]
    w6x_np, split_x = _score_weights(W_att, a_x)
    w6n_np, split_n = _score_weights(W_att, a_n)

    nc = _build_program(bc, split_n, split_x, n_cores)

    def cful(split):
        v = np.concatenate([np.ones(split), -np.ones(HW6 - split)])
        return np.repeat(v[None, :].astype(np.float16), 128, axis=0)

    p = np.arange(128)
    psel4_np = np.equal.outer(p // 32, np.arange(4)).astype(np.float32)
    bsel4_np = np.equal.outer(np.arange(4), p // 32).astype(np.float32)
    bselx_np = np.equal.outer(p % 4, p // 32).astype(np.float32)
    selm_np = np.equal.outer(p // 4, np.arange(NB)).astype(np.float32)
    mask4_np = np.equal.outer(p // 32, np.arange(4)).astype(np.float16)

    shared = {
        "w6n": w6n_np.astype(np.float16), "w6x": w6x_np.astype(np.float16),
        "cfn": cful(split_n), "cfx": cful(split_x),
        "wfcx": W_fcx.astype(np.float16), "wfcn": W_fcn.astype(np.float16),
        "psel4": psel4_np, "bsel4": bsel4_np, "bselx": bselx_np,
        "selm": selm_np, "mask4": mask4_np,
    }

    rows_c = bc * NB
    tiles_c = rows_c // 128
    in_maps = []
    for c in range(n_cores):
        sl = neibs[c * rows_c:(c + 1) * rows_c]
        neT_np = np.ascontiguousarray(sl.T).astype(np.float16)
        neN_np = np.ascontiguousarray(
            sl.reshape(tiles_c, 128, D).transpose(1, 0, 2).reshape(128, rows_c)
        ).astype(np.float16)
        xT_np = np.ascontiguousarray(x[c * bc:(c + 1) * bc].T).astype(np.float16)
        in_maps.append({
            "neT": neT_np, "neN": neN_np, "xT": xT_np, **shared,
        })
    res = run_bass_kernel_spmd(nc, in_maps, core_ids=list(range(n_cores)),
                               **TRACE_OPTS)
    LAST_RESULT[0] = res
    return np.concatenate([res.results[c]["out"] for c in range(n_cores)], axis=0)
